# revision 6
# baseline (speedup 1.0000x reference)
"""Multi-head attention (B=4, S=2048, D=1024, H=16) on 8 TRN2 NeuronCores.

Sharding: batch x head-group (4 batches x 2 groups of 8 heads).  Each core:
  x_b [2048,1024], wq/wk/wv column-slice [1024,512], wo row-slice [512,1024]
  -> partial y [2048,1024]; host sums the two head-group partials per batch
  and adds the folded biases (bo + bv @ wo).

Per-core dataflow (all SBUF-resident, flash-style attention):
  A. x -> xT via PE transposes                      [8 x (128, 2048)]
  B. QT = wq.T @ xT + bq ; KT likewise ; V = x @ wv (+ ones column)
  C. per head-pair, per 512-q block, per 128-k tile:
       L^T = KTh_tile.T @ QTh   (K=64 row-tiled pair, auto tile_position)
       E^T = exp(L^T / 8)       (ACT, scale fused)
       U[q,0:65] += E^T_tile.T @ [V_h | 1]  (PSUM accum over k tiles)
     then U[:,0:64] / U[:,64] -> attention out per head
  D. U -> UT via PE transposes ; y = UT.T @ wo ; DMA out
"""

import numpy as np

from concourse import bass, tile, mybir
from concourse.vector_clock import ScopedClock

F32 = mybir.dt.float32
AF = mybir.ActivationFunctionType

# dtype knobs: (attention operand dtype, projection operand dtype)
DT_ATT = F32   # QT/KT/V/E/U storage & attention matmul operands
DT_PROJ = F32  # xT / weight operands for QKV + O projections

N_CORES = 8
S = 2048          # sequence length per core (one batch)
D = 1024          # d_model
DK = 512          # head-group width (8 heads x 64)
NST = S // 128    # 16 seq tiles
NKC = D // 128    # 8 d_model tiles
NMT = DK // 128   # 4 head-pair tiles


def _install_drain_patch():
    """walrus in this image rejects >1 sync-wait per instruction (the limit
    varies by instruction struct; 1 is always safe).  Spread excess waits
    over preceding same-engine nops: same program point, identical
    semantics, a few ns of sequencer issue overhead."""
    import bass_rust

    MAXW = 1
    _orig_add = tile.TileContext._add_instruction

    def _add_split(self, inst):
        si = inst.sync_info
        waits = list(si.on_wait) if si is not None and si.on_wait else []
        if len(waits) > MAXW and inst.engine != mybir.EngineType.Unassigned:
            rest, keep = waits[:-MAXW], waits[-MAXW:]
            while rest:
                nop = mybir.InstNoOp(
                    name=self.nc.get_next_instruction_name(), ins=[], outs=[]
                )
                nop.engine = inst.engine
                nop.sync_info = bass_rust.SyncInfo(
                    on_wait=rest[:MAXW], on_update=[]
                )
                rest = rest[MAXW:]
                _orig_add(self, nop)
            si.on_wait = keep
        _orig_add(self, inst)

    tile.TileContext._add_instruction = _add_split

    def _patched(self, tick_clock, wait_clock):
        probe = self.nc.sync.nop(nofuse=True)
        wait_clock.add_sem_waits(
            probe.ins, ScopedClock({None: tick_clock.global_clock})
        )
        waits = list(probe.ins.sync_info.on_wait or []) if probe.ins.sync_info else []
        if len(waits) > 1:
            probe.ins.sync_info.on_wait = waits[:1]
            rest = waits[1:]
            while rest:
                n = self.nc.sync.nop(nofuse=True)
                n.ins.sync_info = bass_rust.SyncInfo(on_wait=rest[:1], on_update=[])
                rest = rest[1:]
        self.nc.sync.drain()
        self.nc.all_engine_barrier()
        assert self.sems is not None
        popped = self.nc._tile_sem_poison_stack.pop()
        assert popped is self._sem_poison
        self.nc.clear_and_free_semaphores(list(self.sems.allocated().values()))
        self.nc.all_engine_barrier()

    tile.TileContext._drain_and_barrier = _patched


_install_drain_patch()


def build_nc():
    nc = bass.Bass("TRN2", target_bir_lowering=False, debug=False, num_devices=1)
    xb = nc.dram_tensor("xb", [S, D], F32, kind="ExternalInput").ap()
    wq = nc.dram_tensor("wq", [D, DK], F32, kind="ExternalInput").ap()
    wk = nc.dram_tensor("wk", [D, DK], F32, kind="ExternalInput").ap()
    wv = nc.dram_tensor("wv", [D, DK], F32, kind="ExternalInput").ap()
    bq = nc.dram_tensor("bq", [DK], F32, kind="ExternalInput").ap()
    bk = nc.dram_tensor("bk", [DK], F32, kind="ExternalInput").ap()
    wo = nc.dram_tensor("wo", [DK, D], F32, kind="ExternalInput").ap()
    y = nc.dram_tensor("y", [S, D], F32, kind="ExternalOutput").ap()

    with tile.TileContext(nc, pool_alloc_mode="queue") as tc:
        _emit(nc, tc, xb, wq, wk, wv, bq, bk, wo, y)
    return nc


def _emit(nc, tc, xb, wq, wk, wv, bq, bk, wo, y):
    from contextlib import ExitStack

    ctx = ExitStack()
    with ctx:
        consts = ctx.enter_context(tc.tile_pool(name="consts", bufs=1))
        ident = consts.tile([128, 128], F32, tag="identf32")
        from concourse.masks import make_identity

        make_identity(nc, ident)
        ident_att = ident
        if DT_ATT != F32:
            ident_att = consts.tile([128, 128], DT_ATT, tag="identatt")
            make_identity(nc, ident_att)

        bq_sb = consts.tile([128, NMT], F32, tag="bq")
        bk_sb = consts.tile([128, NMT], F32, tag="bk")
        nc.sync.dma_start(bq_sb[:], bq.rearrange("(m p) -> p m", p=128))
        nc.sync.dma_start(bk_sb[:], bk.rearrange("(m p) -> p m", p=128))

        # ---- persistent attention-phase tensors ----
        qkv_pool = ctx.enter_context(tc.tile_pool(name="qkv", bufs=1))
        QT = [qkv_pool.tile([128, S], DT_ATT, tag=f"QT{m}", name=f"QT{m}") for m in range(NMT)]
        KT = [qkv_pool.tile([128, S], DT_ATT, tag=f"KT{m}", name=f"KT{m}") for m in range(NMT)]
        # V with ones column: [128, st, h, 65]
        VT = qkv_pool.tile([128, NST, 8, 65], DT_ATT, tag="VT")
        nc.vector.memset(VT[:, :, :, 64:65], 1.0)

        # ---- phase A+B scope: xT + weights ----
        with tc.tile_pool(name="xtp", bufs=1) as xtp, \
             tc.tile_pool(name="xin", bufs=6) as xinp, \
             tc.tile_pool(name="wstage", bufs=1) as wst, \
             tc.tile_pool(name="psAB", bufs=4, space="PSUM") as psab:
            xT = [xtp.tile([128, S], DT_PROJ, tag=f"xT{c}", name=f"xT{c}") for c in range(NKC)]

            # A: load + transpose x
            for stg in range(NST // 4):
                xins = []
                for j in range(4):
                    xi = xinp.tile([128, D], F32, tag="xin")
                    nc.sync.dma_start(xi[:], xb[(stg * 4 + j) * 128:(stg * 4 + j + 1) * 128, :])
                    xins.append(xi)
                for c in range(NKC):
                    pt = psab.tile([128, 4, 128], F32, tag="psA")
                    for j in range(4):
                        nc.tensor.transpose(pt[:, j], xins[j][:, c * 128:(c + 1) * 128], ident)
                    nc.vector.tensor_copy(
                        xT[c][:, stg * 512:(stg + 1) * 512].rearrange("p (a b) -> p a b", a=4),
                        pt[:],
                    )

            # B1: V = x @ wv   (natural layout, no bias)
            wv_sb = wst.tile([128, NKC, 512], DT_PROJ, tag="w")
            for kc in range(NKC):
                nc.sync.dma_start(wv_sb[:, kc], wv[kc * 128:(kc + 1) * 128, :])
            for st in range(NST):
                pv = psab.tile([128, 512], F32, tag="psB")
                for kc in range(NKC):
                    nc.tensor.matmul(
                        pv[:], xT[kc][:, st * 128:(st + 1) * 128], wv_sb[:, kc],
                        start=(kc == 0), stop=(kc == NKC - 1),
                    )
                nc.vector.tensor_copy(
                    VT[:, st, :, 0:64],
                    pv[:].rearrange("p (h d) -> p h d", h=8),
                )

            # B2/B3: QT = wq.T @ xT + bq ; KT likewise
            for (w_dram, b_sb, dst) in ((wq, bq_sb, QT), (wk, bk_sb, KT)):
                w_sb = wst.tile([128, NKC, 512], DT_PROJ, tag="w")
                for kc in range(NKC):
                    nc.sync.dma_start(w_sb[:, kc], w_dram[kc * 128:(kc + 1) * 128, :])
                for mt in range(NMT):
                    pqs = [psab.tile([128, 512], F32, tag="psB", name="psB") for _ in range(4)]
                    for kc in range(NKC):
                        for nb in range(4):
                            nc.tensor.matmul(
                                pqs[nb][:],
                                w_sb[:, kc, mt * 128:(mt + 1) * 128],
                                xT[kc][:, nb * 512:(nb + 1) * 512],
                                start=(kc == 0), stop=(kc == NKC - 1),
                            )
                    for nb in range(4):
                        nc.vector.tensor_scalar_add(
                            dst[mt][:, nb * 512:(nb + 1) * 512], pqs[nb][:],
                            b_sb[:, mt:mt + 1],
                        )

        # ---- phase C: attention ----
        upool = ctx.enter_context(tc.tile_pool(name="usb", bufs=1))
        Ut = [upool.tile([128, 8, 64], DT_ATT, tag=f"U{qg}", name=f"U{qg}") for qg in range(NST)]
        with tc.tile_pool(name="lps", bufs=2, space="PSUM") as lps, \
             tc.tile_pool(name="ups", bufs=4, space="PSUM") as ups, \
             tc.tile_pool(name="epool", bufs=3) as epool, \
             tc.tile_pool(name="rpool", bufs=4) as rpool:
            for p in range(NMT):
                for qb in range(4):
                    u1 = ups.tile([128, 4, 128], F32, tag="ups")
                    u2 = ups.tile([128, 4, 128], F32, tag="ups")
                    for kt in range(NST):
                        L = lps.tile([128, 2, 512], F32, tag="L")
                        for half in range(2):
                            hsl = slice(half * 64, (half + 1) * 64)
                            nc.tensor.matmul(
                                L[:, half],
                                KT[p][hsl, kt * 128:(kt + 1) * 128],
                                QT[p][hsl, qb * 512:(qb + 1) * 512],
                                start=True, stop=True,
                            )
                        E = epool.tile([128, 2, 512], DT_ATT, tag="E")
                        nc.scalar.activation(E[:], L[:], AF.Exp, scale=0.125)
                        for qt in range(4):
                            for half, u in ((0, u1), (1, u2)):
                                # start only on the bank's first write: start=True
                                # marks the WHOLE 2KB zero-region pending, so a
                                # per-qt start would wipe earlier slots' kt=0 term.
                                nc.tensor.matmul(
                                    u[:, qt, 0:65],
                                    E[:, half, qt * 128:(qt + 1) * 128],
                                    VT[:, kt, 2 * p + half, :],
                                    start=(kt == 0 and qt == 0),
                                    stop=(kt == NST - 1 and qt == 3),
                                    skip_group_check=True,
                                )
                    for qt in range(4):
                        qg = qb * 4 + qt
                        for half, u in ((0, u1), (1, u2)):
                            r = rpool.tile([128, 1], F32, tag="r")
                            nc.vector.reciprocal(r[:], u[:, qt, 64:65])
                            nc.vector.tensor_scalar_mul(
                                Ut[qg][:, 2 * p + half, :], u[:, qt, 0:64], r[:],
                            )

        # ---- phase D: U -> UT, y = UT.T @ wo ----
        with tc.tile_pool(name="utp", bufs=1) as utp, \
             tc.tile_pool(name="wop", bufs=1) as wop, \
             tc.tile_pool(name="ystage", bufs=3) as ysp, \
             tc.tile_pool(name="psD", bufs=2, space="PSUM") as psd, \
             tc.tile_pool(name="psO", bufs=2, space="PSUM") as pso:
            UT = [utp.tile([128, S], DT_ATT, tag=f"UT{k}", name=f"UT{k}") for k in range(NMT)]
            for kc2 in range(NMT):
                for qtg in range(4):
                    pt = psd.tile([128, 4, 128], DT_ATT, tag="psD")
                    for j in range(4):
                        nc.tensor.transpose(
                            pt[:, j],
                            Ut[qtg * 4 + j][:, 2 * kc2:2 * kc2 + 2, :],
                            ident_att,
                        )
                    nc.vector.tensor_copy(
                        UT[kc2][:, qtg * 512:(qtg + 1) * 512].rearrange("p (a b) -> p a b", a=4),
                        pt[:],
                    )
            wo_sb = wop.tile([128, NMT, D], DT_PROJ, tag="wo")
            for kc2 in range(NMT):
                nc.sync.dma_start(wo_sb[:, kc2], wo[kc2 * 128:(kc2 + 1) * 128, :])
            for qt in range(NST):
                yp = pso.tile([128, 2, 512], F32, tag="psO")
                for kc2 in range(NMT):
                    for n in range(2):
                        nc.tensor.matmul(
                            yp[:, n],
                            UT[kc2][:, qt * 128:(qt + 1) * 128],
                            wo_sb[:, kc2, n * 512:(n + 1) * 512],
                            start=(kc2 == 0), stop=(kc2 == NMT - 1),
                            skip_group_check=True,
                        )
                ys = ysp.tile([128, D], F32, tag="ys")
                nc.vector.tensor_copy(ys[:].rearrange("p (a b) -> p a b", a=2), yp[:])
                nc.sync.dma_start(y[qt * 128:(qt + 1) * 128, :], ys[:])


_NC_CACHE = None


def _get_nc():
    global _NC_CACHE
    if _NC_CACHE is None:
        _NC_CACHE = build_nc()
    return _NC_CACHE


def kernel(x, wq, bq, wk, bk, wv, bv, wo, bo):
    from concourse.bass_utils import run_bass_kernel_spmd

    x = np.ascontiguousarray(np.asarray(x, np.float32))
    wq, bq = np.asarray(wq, np.float32), np.asarray(bq, np.float32)
    wk, bk = np.asarray(wk, np.float32), np.asarray(bk, np.float32)
    wv, bv = np.asarray(wv, np.float32), np.asarray(bv, np.float32)
    wo, bo = np.asarray(wo, np.float32), np.asarray(bo, np.float32)

    in_maps = []
    for core in range(N_CORES):
        b, hg = core // 2, core % 2
        sl = slice(hg * DK, (hg + 1) * DK)
        in_maps.append({
            "xb": np.ascontiguousarray(x[b]),
            "wq": np.ascontiguousarray(wq[:, sl]),
            "wk": np.ascontiguousarray(wk[:, sl]),
            "wv": np.ascontiguousarray(wv[:, sl]),
            "bq": np.ascontiguousarray(bq[sl]),
            "bk": np.ascontiguousarray(bk[sl]),
            "wo": np.ascontiguousarray(wo[sl, :]),
        })

    nc = _get_nc()
    res = run_bass_kernel_spmd(nc, in_maps, core_ids=list(range(N_CORES)))

    extra = (bo + bv @ wo).astype(np.float32)
    out = np.empty((4, S, D), np.float32)
    for b in range(4):
        out[b] = res.results[2 * b]["y"] + res.results[2 * b + 1]["y"] + extra
    return out


# revision 8
# speedup vs baseline: 1.7706x; 1.7706x over previous
"""Multi-head attention (B=4, S=2048, D=1024, H=16) on 8 TRN2 NeuronCores.

Sharding: batch x head-group (4 batches x 2 groups of 8 heads).  Each core:
  x_b [2048,1024], wq/wk/wv column-slice [1024,512], wo row-slice [512,1024]
  -> partial y [2048,1024]; host sums the two head-group partials per batch
  and adds the folded biases (bo + bv @ wo).

Per-core dataflow (all SBUF-resident, flash-style attention):
  A. x -> xT via PE transposes                      [8 x (128, 2048)]
  B. QT = wq.T @ xT + bq ; KT likewise ; V = x @ wv (+ ones column)
  C. per head-pair, per 512-q block, per 128-k tile:
       L^T = KTh_tile.T @ QTh   (K=64 row-tiled pair, auto tile_position)
       E^T = exp(L^T / 8)       (ACT, scale fused)
       U[q,0:65] += E^T_tile.T @ [V_h | 1]  (PSUM accum over k tiles)
     then U[:,0:64] / U[:,64] -> attention out per head
  D. U -> UT via PE transposes ; y = UT.T @ wo ; DMA out
"""

import numpy as np

from concourse import bass, tile, mybir
from concourse.vector_clock import ScopedClock

F32 = mybir.dt.float32
AF = mybir.ActivationFunctionType

# dtype knobs: (attention operand dtype, projection operand dtype)
DT_ATT = mybir.dt.bfloat16   # QT/KT/V/E/U storage & attention matmul operands
DT_PROJ = F32                # xT / weight operands for QKV projections

N_CORES = 8
S = 2048          # sequence length per core (one batch)
D = 1024          # d_model
DK = 512          # head-group width (8 heads x 64)
NST = S // 128    # 16 seq tiles
NKC = D // 128    # 8 d_model tiles
NMT = DK // 128   # 4 head-pair tiles


def _install_drain_patch():
    """walrus in this image rejects >1 sync-wait per instruction (the limit
    varies by instruction struct; 1 is always safe).  Spread excess waits
    over preceding same-engine nops: same program point, identical
    semantics, a few ns of sequencer issue overhead."""
    import bass_rust

    MAXW = 1
    _orig_add = tile.TileContext._add_instruction

    def _add_split(self, inst):
        si = inst.sync_info
        waits = list(si.on_wait) if si is not None and si.on_wait else []
        if len(waits) > MAXW and inst.engine != mybir.EngineType.Unassigned:
            rest, keep = waits[:-MAXW], waits[-MAXW:]
            while rest:
                nop = mybir.InstNoOp(
                    name=self.nc.get_next_instruction_name(), ins=[], outs=[]
                )
                nop.engine = inst.engine
                nop.sync_info = bass_rust.SyncInfo(
                    on_wait=rest[:MAXW], on_update=[]
                )
                rest = rest[MAXW:]
                _orig_add(self, nop)
            si.on_wait = keep
        _orig_add(self, inst)

    tile.TileContext._add_instruction = _add_split

    def _patched(self, tick_clock, wait_clock):
        probe = self.nc.sync.nop(nofuse=True)
        wait_clock.add_sem_waits(
            probe.ins, ScopedClock({None: tick_clock.global_clock})
        )
        waits = list(probe.ins.sync_info.on_wait or []) if probe.ins.sync_info else []
        if len(waits) > 1:
            probe.ins.sync_info.on_wait = waits[:1]
            rest = waits[1:]
            while rest:
                n = self.nc.sync.nop(nofuse=True)
                n.ins.sync_info = bass_rust.SyncInfo(on_wait=rest[:1], on_update=[])
                rest = rest[1:]
        self.nc.sync.drain()
        self.nc.all_engine_barrier()
        assert self.sems is not None
        popped = self.nc._tile_sem_poison_stack.pop()
        assert popped is self._sem_poison
        self.nc.clear_and_free_semaphores(list(self.sems.allocated().values()))
        self.nc.all_engine_barrier()

    tile.TileContext._drain_and_barrier = _patched


_install_drain_patch()


def build_nc():
    nc = bass.Bass("TRN2", target_bir_lowering=False, debug=False, num_devices=1)
    xb = nc.dram_tensor("xb", [S, D], F32, kind="ExternalInput").ap()
    wq = nc.dram_tensor("wq", [D, DK], F32, kind="ExternalInput").ap()
    wk = nc.dram_tensor("wk", [D, DK], F32, kind="ExternalInput").ap()
    wv = nc.dram_tensor("wv", [D, DK], F32, kind="ExternalInput").ap()
    bq = nc.dram_tensor("bq", [DK], F32, kind="ExternalInput").ap()
    bk = nc.dram_tensor("bk", [DK], F32, kind="ExternalInput").ap()
    wo = nc.dram_tensor("wo", [DK, D], F32, kind="ExternalInput").ap()
    y = nc.dram_tensor("y", [S, D], F32, kind="ExternalOutput").ap()

    with tile.TileContext(nc, pool_alloc_mode="queue") as tc:
        _emit(nc, tc, xb, wq, wk, wv, bq, bk, wo, y)
    return nc


def _emit(nc, tc, xb, wq, wk, wv, bq, bk, wo, y):
    from contextlib import ExitStack

    ctx = ExitStack()
    with ctx:
        consts = ctx.enter_context(tc.tile_pool(name="consts", bufs=1))
        ident = consts.tile([128, 128], F32, tag="identf32")
        from concourse.masks import make_identity

        make_identity(nc, ident)
        ident_att = ident
        if DT_ATT != F32:
            ident_att = consts.tile([128, 128], DT_ATT, tag="identatt")
            make_identity(nc, ident_att)

        bq_sb = consts.tile([128, NMT], F32, tag="bq")
        bk_sb = consts.tile([128, NMT], F32, tag="bk")
        nc.sync.dma_start(bq_sb[:], bq.rearrange("(m p) -> p m", p=128))
        nc.sync.dma_start(bk_sb[:], bk.rearrange("(m p) -> p m", p=128))

        # ---- persistent attention-phase tensors ----
        qkv_pool = ctx.enter_context(tc.tile_pool(name="qkv", bufs=1))
        QT = [qkv_pool.tile([128, S], DT_ATT, tag=f"QT{m}", name=f"QT{m}") for m in range(NMT)]
        KT = [qkv_pool.tile([128, S], DT_ATT, tag=f"KT{m}", name=f"KT{m}") for m in range(NMT)]
        # V with ones column: [128, st, h, 65]
        VT = qkv_pool.tile([128, NST, 8, 65], DT_ATT, tag="VT")
        nc.vector.memset(VT[:, :, :, 64:65], 1.0)

        # ---- phase A+B scope: xT + weights ----
        with tc.tile_pool(name="xtp", bufs=1) as xtp, \
             tc.tile_pool(name="xin", bufs=6) as xinp, \
             tc.tile_pool(name="wstage", bufs=1) as wst, \
             tc.tile_pool(name="psAB", bufs=4, space="PSUM") as psab:
            xT = [xtp.tile([128, S], DT_PROJ, tag=f"xT{c}", name=f"xT{c}") for c in range(NKC)]

            # A: load + transpose x
            for stg in range(NST // 4):
                xins = []
                for j in range(4):
                    xi = xinp.tile([128, D], F32, tag="xin")
                    nc.sync.dma_start(xi[:], xb[(stg * 4 + j) * 128:(stg * 4 + j + 1) * 128, :])
                    xins.append(xi)
                for c in range(NKC):
                    pt = psab.tile([128, 4, 128], F32, tag="psA")
                    for j in range(4):
                        nc.tensor.transpose(pt[:, j], xins[j][:, c * 128:(c + 1) * 128], ident)
                    nc.vector.tensor_copy(
                        xT[c][:, stg * 512:(stg + 1) * 512].rearrange("p (a b) -> p a b", a=4),
                        pt[:],
                    )

            # B1: V = x @ wv   (natural layout, no bias)
            wv_sb = wst.tile([128, NKC, 512], DT_PROJ, tag="w")
            for kc in range(NKC):
                nc.sync.dma_start(wv_sb[:, kc], wv[kc * 128:(kc + 1) * 128, :])
            for st in range(NST):
                pv = psab.tile([128, 512], F32, tag="psB")
                for kc in range(NKC):
                    nc.tensor.matmul(
                        pv[:], xT[kc][:, st * 128:(st + 1) * 128], wv_sb[:, kc],
                        start=(kc == 0), stop=(kc == NKC - 1),
                    )
                nc.vector.tensor_copy(
                    VT[:, st, :, 0:64],
                    pv[:].rearrange("p (h d) -> p h d", h=8),
                )

            # B2/B3: QT = wq.T @ xT + bq ; KT likewise
            for (w_dram, b_sb, dst) in ((wq, bq_sb, QT), (wk, bk_sb, KT)):
                w_sb = wst.tile([128, NKC, 512], DT_PROJ, tag="w")
                for kc in range(NKC):
                    nc.sync.dma_start(w_sb[:, kc], w_dram[kc * 128:(kc + 1) * 128, :])
                for mt in range(NMT):
                    pqs = [psab.tile([128, 512], F32, tag="psB", name="psB") for _ in range(4)]
                    for kc in range(NKC):
                        for nb in range(4):
                            nc.tensor.matmul(
                                pqs[nb][:],
                                w_sb[:, kc, mt * 128:(mt + 1) * 128],
                                xT[kc][:, nb * 512:(nb + 1) * 512],
                                start=(kc == 0), stop=(kc == NKC - 1),
                            )
                    for nb in range(4):
                        nc.vector.tensor_scalar_add(
                            dst[mt][:, nb * 512:(nb + 1) * 512], pqs[nb][:],
                            b_sb[:, mt:mt + 1],
                        )

        # ---- phase C: attention ----
        upool = ctx.enter_context(tc.tile_pool(name="usb", bufs=1))
        Ut = [upool.tile([128, 8, 64], DT_ATT, tag=f"U{qg}", name=f"U{qg}") for qg in range(NST)]
        with tc.tile_pool(name="lps", bufs=2, space="PSUM") as lps, \
             tc.tile_pool(name="ups", bufs=4, space="PSUM") as ups, \
             tc.tile_pool(name="epool", bufs=3) as epool, \
             tc.tile_pool(name="rpool", bufs=4) as rpool:
            for p in range(NMT):
                for qb in range(4):
                    u1 = ups.tile([128, 4, 128], F32, tag="ups")
                    u2 = ups.tile([128, 4, 128], F32, tag="ups")
                    for kt in range(NST):
                        L = lps.tile([128, 2, 512], F32, tag="L")
                        for half in range(2):
                            hsl = slice(half * 64, (half + 1) * 64)
                            nc.tensor.matmul(
                                L[:, half],
                                KT[p][hsl, kt * 128:(kt + 1) * 128],
                                QT[p][hsl, qb * 512:(qb + 1) * 512],
                                start=True, stop=True,
                            )
                        E = epool.tile([128, 2, 512], DT_ATT, tag="E")
                        nc.scalar.activation(E[:], L[:], AF.Exp, scale=0.125)
                        for qt in range(4):
                            for half, u in ((0, u1), (1, u2)):
                                # start only on the bank's first write: start=True
                                # marks the WHOLE 2KB zero-region pending, so a
                                # per-qt start would wipe earlier slots' kt=0 term.
                                nc.tensor.matmul(
                                    u[:, qt, 0:65],
                                    E[:, half, qt * 128:(qt + 1) * 128],
                                    VT[:, kt, 2 * p + half, :],
                                    start=(kt == 0 and qt == 0),
                                    stop=(kt == NST - 1 and qt == 3),
                                    skip_group_check=True,
                                )
                    for qt in range(4):
                        qg = qb * 4 + qt
                        for half, u in ((0, u1), (1, u2)):
                            r = rpool.tile([128, 1], F32, tag="r")
                            nc.vector.reciprocal(r[:], u[:, qt, 64:65])
                            nc.vector.tensor_scalar_mul(
                                Ut[qg][:, 2 * p + half, :], u[:, qt, 0:64], r[:],
                            )

        # ---- phase D: U -> UT, y = UT.T @ wo ----
        with tc.tile_pool(name="utp", bufs=1) as utp, \
             tc.tile_pool(name="wop", bufs=1) as wop, \
             tc.tile_pool(name="ystage", bufs=3) as ysp, \
             tc.tile_pool(name="psD", bufs=2, space="PSUM") as psd, \
             tc.tile_pool(name="psO", bufs=2, space="PSUM") as pso:
            UT = [utp.tile([128, S], DT_ATT, tag=f"UT{k}", name=f"UT{k}") for k in range(NMT)]
            for kc2 in range(NMT):
                for qtg in range(4):
                    pt = psd.tile([128, 4, 128], DT_ATT, tag="psD")
                    for j in range(4):
                        nc.tensor.transpose(
                            pt[:, j],
                            Ut[qtg * 4 + j][:, 2 * kc2:2 * kc2 + 2, :],
                            ident_att,
                        )
                    nc.vector.tensor_copy(
                        UT[kc2][:, qtg * 512:(qtg + 1) * 512].rearrange("p (a b) -> p a b", a=4),
                        pt[:],
                    )
            wo_sb = wop.tile([128, NMT, D], DT_ATT, tag="wo")
            if DT_ATT == F32:
                for kc2 in range(NMT):
                    nc.sync.dma_start(wo_sb[:, kc2], wo[kc2 * 128:(kc2 + 1) * 128, :])
            else:
                wo_f32 = wop.tile([128, NMT, D], F32, tag="wof")
                for kc2 in range(NMT):
                    nc.sync.dma_start(wo_f32[:, kc2], wo[kc2 * 128:(kc2 + 1) * 128, :])
                nc.vector.tensor_copy(wo_sb[:], wo_f32[:])
            for qt in range(NST):
                yp = pso.tile([128, 2, 512], F32, tag="psO")
                for kc2 in range(NMT):
                    for n in range(2):
                        nc.tensor.matmul(
                            yp[:, n],
                            UT[kc2][:, qt * 128:(qt + 1) * 128],
                            wo_sb[:, kc2, n * 512:(n + 1) * 512],
                            start=(kc2 == 0), stop=(kc2 == NMT - 1),
                            skip_group_check=True,
                        )
                ys = ysp.tile([128, D], F32, tag="ys")
                nc.vector.tensor_copy(ys[:].rearrange("p (a b) -> p a b", a=2), yp[:])
                nc.sync.dma_start(y[qt * 128:(qt + 1) * 128, :], ys[:])


_NC_CACHE = None


def _get_nc():
    global _NC_CACHE
    if _NC_CACHE is None:
        _NC_CACHE = build_nc()
    return _NC_CACHE


def kernel(x, wq, bq, wk, bk, wv, bv, wo, bo):
    from concourse.bass_utils import run_bass_kernel_spmd

    x = np.ascontiguousarray(np.asarray(x, np.float32))
    wq, bq = np.asarray(wq, np.float32), np.asarray(bq, np.float32)
    wk, bk = np.asarray(wk, np.float32), np.asarray(bk, np.float32)
    wv, bv = np.asarray(wv, np.float32), np.asarray(bv, np.float32)
    wo, bo = np.asarray(wo, np.float32), np.asarray(bo, np.float32)

    in_maps = []
    for core in range(N_CORES):
        b, hg = core // 2, core % 2
        sl = slice(hg * DK, (hg + 1) * DK)
        in_maps.append({
            "xb": np.ascontiguousarray(x[b]),
            "wq": np.ascontiguousarray(wq[:, sl]),
            "wk": np.ascontiguousarray(wk[:, sl]),
            "wv": np.ascontiguousarray(wv[:, sl]),
            "bq": np.ascontiguousarray(bq[sl]),
            "bk": np.ascontiguousarray(bk[sl]),
            "wo": np.ascontiguousarray(wo[sl, :]),
        })

    nc = _get_nc()
    res = run_bass_kernel_spmd(nc, in_maps, core_ids=list(range(N_CORES)))

    extra = (bo + bv @ wo).astype(np.float32)
    out = np.empty((4, S, D), np.float32)
    for b in range(4):
        out[b] = res.results[2 * b]["y"] + res.results[2 * b + 1]["y"] + extra
    return out


# revision 11
# speedup vs baseline: 2.6499x; 1.4966x over previous
"""Multi-head attention (B=4, S=2048, D=1024, H=16) on 8 TRN2 NeuronCores.

Sharding: batch x head-group (4 batches x 2 groups of 8 heads).  Each core:
  x_b [2048,1024], wq/wk/wv column-slice [1024,512], wo row-slice [512,1024]
  -> partial y [2048,1024]; host sums the two head-group partials per batch
  and adds the folded biases (bo + bv @ wo).

Per-core dataflow (all SBUF-resident, flash-style attention):
  A. x -> xT via PE transposes                      [8 x (128, 2048)]
  B. QT = wq.T @ xT + bq ; KT likewise ; V = x @ wv (+ ones column)
  C. per head-pair, per 512-q block, per 128-k tile:
       L^T = KTh_tile.T @ QTh   (K=64 row-tiled pair, auto tile_position)
       E^T = exp(L^T / 8)       (ACT, scale fused)
       U[q,0:65] += E^T_tile.T @ [V_h | 1]  (PSUM accum over k tiles)
     then U[:,0:64] / U[:,64] -> attention out per head
  D. U -> UT via PE transposes ; y = UT.T @ wo ; DMA out
"""

import numpy as np

from concourse import bass, tile, mybir
from concourse.vector_clock import ScopedClock

F32 = mybir.dt.float32
AF = mybir.ActivationFunctionType

# dtype knobs: (attention operand dtype, projection operand dtype)
DT_ATT = mybir.dt.bfloat16   # QT/KT/V/E/U storage & attention matmul operands
DT_PROJ = mybir.dt.bfloat16  # xT / weight operands for QKV projections

N_CORES = 8
S = 2048          # sequence length per core (one batch)
D = 1024          # d_model
DK = 512          # head-group width (8 heads x 64)
NST = S // 128    # 16 seq tiles
NKC = D // 128    # 8 d_model tiles
NMT = DK // 128   # 4 head-pair tiles


def _install_drain_patch():
    """walrus in this image rejects >1 sync-wait per instruction (the limit
    varies by instruction struct; 1 is always safe).  Spread excess waits
    over preceding same-engine nops: same program point, identical
    semantics, a few ns of sequencer issue overhead."""
    import bass_rust

    MAXW = 1
    _orig_add = tile.TileContext._add_instruction

    def _add_split(self, inst):
        si = inst.sync_info
        waits = list(si.on_wait) if si is not None and si.on_wait else []
        if len(waits) > MAXW and inst.engine != mybir.EngineType.Unassigned:
            rest, keep = waits[:-MAXW], waits[-MAXW:]
            while rest:
                nop = mybir.InstNoOp(
                    name=self.nc.get_next_instruction_name(), ins=[], outs=[]
                )
                nop.engine = inst.engine
                nop.sync_info = bass_rust.SyncInfo(
                    on_wait=rest[:MAXW], on_update=[]
                )
                rest = rest[MAXW:]
                _orig_add(self, nop)
            si.on_wait = keep
        _orig_add(self, inst)

    tile.TileContext._add_instruction = _add_split

    def _patched(self, tick_clock, wait_clock):
        probe = self.nc.sync.nop(nofuse=True)
        wait_clock.add_sem_waits(
            probe.ins, ScopedClock({None: tick_clock.global_clock})
        )
        waits = list(probe.ins.sync_info.on_wait or []) if probe.ins.sync_info else []
        if len(waits) > 1:
            probe.ins.sync_info.on_wait = waits[:1]
            rest = waits[1:]
            while rest:
                n = self.nc.sync.nop(nofuse=True)
                n.ins.sync_info = bass_rust.SyncInfo(on_wait=rest[:1], on_update=[])
                rest = rest[1:]
        self.nc.sync.drain()
        self.nc.all_engine_barrier()
        assert self.sems is not None
        popped = self.nc._tile_sem_poison_stack.pop()
        assert popped is self._sem_poison
        self.nc.clear_and_free_semaphores(list(self.sems.allocated().values()))
        self.nc.all_engine_barrier()

    tile.TileContext._drain_and_barrier = _patched


_install_drain_patch()


def build_nc():
    nc = bass.Bass("TRN2", target_bir_lowering=False, debug=False, num_devices=1)
    xb = nc.dram_tensor("xb", [S, D], F32, kind="ExternalInput").ap()
    wq = nc.dram_tensor("wq", [D, DK], F32, kind="ExternalInput").ap()
    wk = nc.dram_tensor("wk", [D, DK], F32, kind="ExternalInput").ap()
    wv = nc.dram_tensor("wv", [D, DK], F32, kind="ExternalInput").ap()
    bq = nc.dram_tensor("bq", [DK], F32, kind="ExternalInput").ap()
    bk = nc.dram_tensor("bk", [DK], F32, kind="ExternalInput").ap()
    wo = nc.dram_tensor("wo", [DK, D], F32, kind="ExternalInput").ap()
    y = nc.dram_tensor("y", [S, D], F32, kind="ExternalOutput").ap()

    with tile.TileContext(nc, pool_alloc_mode="queue") as tc:
        _emit(nc, tc, xb, wq, wk, wv, bq, bk, wo, y)
    return nc


def _emit(nc, tc, xb, wq, wk, wv, bq, bk, wo, y):
    from contextlib import ExitStack

    ctx = ExitStack()
    with ctx:
        consts = ctx.enter_context(tc.tile_pool(name="consts", bufs=1))
        ident = consts.tile([128, 128], F32, tag="identf32")
        from concourse.masks import make_identity

        make_identity(nc, ident)
        ident_att = ident
        if DT_ATT != F32:
            ident_att = consts.tile([128, 128], DT_ATT, tag="identatt")
            make_identity(nc, ident_att)

        bq_sb = consts.tile([128, NMT], F32, tag="bq")
        bk_sb = consts.tile([128, NMT], F32, tag="bk")
        nc.sync.dma_start(bq_sb[:], bq.rearrange("(m p) -> p m", p=128))
        nc.sync.dma_start(bk_sb[:], bk.rearrange("(m p) -> p m", p=128))

        # ---- persistent attention-phase tensors ----
        qkv_pool = ctx.enter_context(tc.tile_pool(name="qkv", bufs=1))
        QT = [qkv_pool.tile([128, S], DT_ATT, tag=f"QT{m}", name=f"QT{m}") for m in range(NMT)]
        KT = [qkv_pool.tile([128, S], DT_ATT, tag=f"KT{m}", name=f"KT{m}") for m in range(NMT)]
        # V with ones column: [128, st, h, 65]
        VT = qkv_pool.tile([128, NST, 8, 65], DT_ATT, tag="VT")
        nc.vector.memset(VT[:, :, :, 64:65], 1.0)

        # ---- phase A+B scope: xT + weights ----
        with tc.tile_pool(name="xtp", bufs=1) as xtp, \
             tc.tile_pool(name="xin", bufs=6) as xinp, \
             tc.tile_pool(name="wstage", bufs=1) as wst, \
             tc.tile_pool(name="psAB", bufs=4, space="PSUM") as psab:
            xT = [xtp.tile([128, S], DT_PROJ, tag=f"xT{c}", name=f"xT{c}") for c in range(NKC)]

            # A: load + transpose x
            for stg in range(NST // 4):
                xins = []
                for j in range(4):
                    xi = xinp.tile([128, D], F32, tag="xin")
                    nc.sync.dma_start(xi[:], xb[(stg * 4 + j) * 128:(stg * 4 + j + 1) * 128, :])
                    xins.append(xi)
                for c in range(NKC):
                    pt = psab.tile([128, 4, 128], F32, tag="psA")
                    for j in range(4):
                        nc.tensor.transpose(pt[:, j], xins[j][:, c * 128:(c + 1) * 128], ident)
                    nc.vector.tensor_copy(
                        xT[c][:, stg * 512:(stg + 1) * 512].rearrange("p (a b) -> p a b", a=4),
                        pt[:],
                    )

            def load_w(w_dram):
                w_sb = wst.tile([128, NKC, 512], DT_PROJ, tag="w", name="w_sb")
                if DT_PROJ == F32:
                    for kc in range(NKC):
                        nc.sync.dma_start(w_sb[:, kc], w_dram[kc * 128:(kc + 1) * 128, :])
                else:
                    w_f32 = wst.tile([128, NKC, 512], F32, tag="wf32", name="w_f32")
                    for kc in range(NKC):
                        nc.sync.dma_start(w_f32[:, kc], w_dram[kc * 128:(kc + 1) * 128, :])
                    nc.vector.tensor_copy(w_sb[:], w_f32[:])
                return w_sb

            # B1: V = x @ wv   (natural layout, no bias)
            wv_sb = load_w(wv)
            for st in range(NST):
                pv = psab.tile([128, 512], F32, tag="psB")
                for kc in range(NKC):
                    nc.tensor.matmul(
                        pv[:], xT[kc][:, st * 128:(st + 1) * 128], wv_sb[:, kc],
                        start=(kc == 0), stop=(kc == NKC - 1),
                    )
                nc.vector.tensor_copy(
                    VT[:, st, :, 0:64],
                    pv[:].rearrange("p (h d) -> p h d", h=8),
                )

            # B2/B3: QT = wq.T @ xT + bq ; KT likewise
            for (w_dram, b_sb, dst) in ((wq, bq_sb, QT), (wk, bk_sb, KT)):
                w_sb = load_w(w_dram)
                for mt in range(NMT):
                    pqs = [psab.tile([128, 512], F32, tag="psB", name="psB") for _ in range(4)]
                    for kc in range(NKC):
                        for nb in range(4):
                            nc.tensor.matmul(
                                pqs[nb][:],
                                w_sb[:, kc, mt * 128:(mt + 1) * 128],
                                xT[kc][:, nb * 512:(nb + 1) * 512],
                                start=(kc == 0), stop=(kc == NKC - 1),
                            )
                    for nb in range(4):
                        nc.vector.tensor_scalar_add(
                            dst[mt][:, nb * 512:(nb + 1) * 512], pqs[nb][:],
                            b_sb[:, mt:mt + 1],
                        )

        # ---- phase C: attention ----
        upool = ctx.enter_context(tc.tile_pool(name="usb", bufs=1))
        Ut = [upool.tile([128, 8, 64], DT_ATT, tag=f"U{qg}", name=f"U{qg}") for qg in range(NST)]
        with tc.tile_pool(name="lps", bufs=2, space="PSUM") as lps, \
             tc.tile_pool(name="ups", bufs=4, space="PSUM") as ups, \
             tc.tile_pool(name="epool", bufs=3) as epool, \
             tc.tile_pool(name="rpool", bufs=4) as rpool:
            for p in range(NMT):
                for qb in range(4):
                    u1 = ups.tile([128, 4, 128], F32, tag="ups")
                    u2 = ups.tile([128, 4, 128], F32, tag="ups")
                    for kt in range(NST):
                        L = lps.tile([128, 2, 512], F32, tag="L")
                        for half in range(2):
                            hsl = slice(half * 64, (half + 1) * 64)
                            nc.tensor.matmul(
                                L[:, half],
                                KT[p][hsl, kt * 128:(kt + 1) * 128],
                                QT[p][hsl, qb * 512:(qb + 1) * 512],
                                start=True, stop=True,
                            )
                        E = epool.tile([128, 2, 512], DT_ATT, tag="E")
                        nc.scalar.activation(E[:], L[:], AF.Exp, scale=0.125)
                        for qt in range(4):
                            for half, u in ((0, u1), (1, u2)):
                                # start only on the bank's first write: start=True
                                # marks the WHOLE 2KB zero-region pending, so a
                                # per-qt start would wipe earlier slots' kt=0 term.
                                nc.tensor.matmul(
                                    u[:, qt, 0:65],
                                    E[:, half, qt * 128:(qt + 1) * 128],
                                    VT[:, kt, 2 * p + half, :],
                                    start=(kt == 0 and qt == 0),
                                    stop=(kt == NST - 1 and qt == 3),
                                    skip_group_check=True,
                                )
                    for qt in range(4):
                        qg = qb * 4 + qt
                        for half, u in ((0, u1), (1, u2)):
                            r = rpool.tile([128, 1], F32, tag="r")
                            nc.vector.reciprocal(r[:], u[:, qt, 64:65])
                            nc.vector.tensor_scalar_mul(
                                Ut[qg][:, 2 * p + half, :], u[:, qt, 0:64], r[:],
                            )

        # ---- phase D: U -> UT, y = UT.T @ wo ----
        with tc.tile_pool(name="utp", bufs=1) as utp, \
             tc.tile_pool(name="wop", bufs=1) as wop, \
             tc.tile_pool(name="ystage", bufs=3) as ysp, \
             tc.tile_pool(name="psD", bufs=2, space="PSUM") as psd, \
             tc.tile_pool(name="psO", bufs=2, space="PSUM") as pso:
            UT = [utp.tile([128, S], DT_ATT, tag=f"UT{k}", name=f"UT{k}") for k in range(NMT)]
            for kc2 in range(NMT):
                for qtg in range(4):
                    pt = psd.tile([128, 4, 128], DT_ATT, tag="psD")
                    for j in range(4):
                        nc.tensor.transpose(
                            pt[:, j],
                            Ut[qtg * 4 + j][:, 2 * kc2:2 * kc2 + 2, :],
                            ident_att,
                        )
                    nc.vector.tensor_copy(
                        UT[kc2][:, qtg * 512:(qtg + 1) * 512].rearrange("p (a b) -> p a b", a=4),
                        pt[:],
                    )
            wo_sb = wop.tile([128, NMT, D], DT_ATT, tag="wo")
            if DT_ATT == F32:
                for kc2 in range(NMT):
                    nc.sync.dma_start(wo_sb[:, kc2], wo[kc2 * 128:(kc2 + 1) * 128, :])
            else:
                wo_f32 = wop.tile([128, NMT, D], F32, tag="wof")
                for kc2 in range(NMT):
                    nc.sync.dma_start(wo_f32[:, kc2], wo[kc2 * 128:(kc2 + 1) * 128, :])
                nc.vector.tensor_copy(wo_sb[:], wo_f32[:])
            for qt in range(NST):
                yp = pso.tile([128, 2, 512], F32, tag="psO")
                for kc2 in range(NMT):
                    for n in range(2):
                        nc.tensor.matmul(
                            yp[:, n],
                            UT[kc2][:, qt * 128:(qt + 1) * 128],
                            wo_sb[:, kc2, n * 512:(n + 1) * 512],
                            start=(kc2 == 0), stop=(kc2 == NMT - 1),
                            skip_group_check=True,
                        )
                ys = ysp.tile([128, D], F32, tag="ys")
                nc.vector.tensor_copy(ys[:].rearrange("p (a b) -> p a b", a=2), yp[:])
                nc.sync.dma_start(y[qt * 128:(qt + 1) * 128, :], ys[:])


_NC_CACHE = None


def _get_nc():
    global _NC_CACHE
    if _NC_CACHE is None:
        _NC_CACHE = build_nc()
    return _NC_CACHE


def kernel(x, wq, bq, wk, bk, wv, bv, wo, bo):
    from concourse.bass_utils import run_bass_kernel_spmd

    x = np.ascontiguousarray(np.asarray(x, np.float32))
    wq, bq = np.asarray(wq, np.float32), np.asarray(bq, np.float32)
    wk, bk = np.asarray(wk, np.float32), np.asarray(bk, np.float32)
    wv, bv = np.asarray(wv, np.float32), np.asarray(bv, np.float32)
    wo, bo = np.asarray(wo, np.float32), np.asarray(bo, np.float32)

    in_maps = []
    for core in range(N_CORES):
        b, hg = core // 2, core % 2
        sl = slice(hg * DK, (hg + 1) * DK)
        in_maps.append({
            "xb": np.ascontiguousarray(x[b]),
            "wq": np.ascontiguousarray(wq[:, sl]),
            "wk": np.ascontiguousarray(wk[:, sl]),
            "wv": np.ascontiguousarray(wv[:, sl]),
            "bq": np.ascontiguousarray(bq[sl]),
            "bk": np.ascontiguousarray(bk[sl]),
            "wo": np.ascontiguousarray(wo[sl, :]),
        })

    nc = _get_nc()
    res = run_bass_kernel_spmd(nc, in_maps, core_ids=list(range(N_CORES)))

    extra = (bo + bv @ wo).astype(np.float32)
    out = np.empty((4, S, D), np.float32)
    for b in range(4):
        out[b] = res.results[2 * b]["y"] + res.results[2 * b + 1]["y"] + extra
    return out


# revision 12
# speedup vs baseline: 3.3294x; 1.2564x over previous
"""Multi-head attention (B=4, S=2048, D=1024, H=16) on 8 TRN2 NeuronCores.

Sharding: batch x head-group (4 batches x 2 groups of 8 heads).  Each core:
  x_b [2048,1024], wq/wk/wv column-slice [1024,512], wo row-slice [512,1024]
  -> partial y [2048,1024]; host sums the two head-group partials per batch
  and adds the folded biases (bo + bv @ wo).

Per-core dataflow (all SBUF-resident, flash-style attention):
  A. x -> xT via PE transposes                      [8 x (128, 2048)]
  B. QT = wq.T @ xT + bq ; KT likewise ; V = x @ wv (+ ones column)
  C. per head-pair, per 512-q block, per 128-k tile:
       L^T = KTh_tile.T @ QTh   (K=64 row-tiled pair, auto tile_position)
       E^T = exp(L^T / 8)       (ACT, scale fused)
       U[q,0:65] += E^T_tile.T @ [V_h | 1]  (PSUM accum over k tiles)
     then U[:,0:64] / U[:,64] -> attention out per head
  D. U -> UT via PE transposes ; y = UT.T @ wo ; DMA out
"""

import numpy as np

from concourse import bass, tile, mybir
from concourse.vector_clock import ScopedClock

F32 = mybir.dt.float32
AF = mybir.ActivationFunctionType

# dtype knobs: (attention operand dtype, projection operand dtype)
DT_ATT = mybir.dt.bfloat16   # QT/KT/V/E/U storage & attention matmul operands
DT_PROJ = mybir.dt.bfloat16  # xT / weight operands for QKV projections

N_CORES = 8
S = 2048          # sequence length per core (one batch)
D = 1024          # d_model
DK = 512          # head-group width (8 heads x 64)
NST = S // 128    # 16 seq tiles
NKC = D // 128    # 8 d_model tiles
NMT = DK // 128   # 4 head-pair tiles


def _install_drain_patch():
    """walrus in this image rejects >1 sync-wait per instruction (the limit
    varies by instruction struct; 1 is always safe).  Spread excess waits
    over preceding same-engine nops: same program point, identical
    semantics, a few ns of sequencer issue overhead."""
    import bass_rust

    MAXW = 1
    _orig_add = tile.TileContext._add_instruction

    def _add_split(self, inst):
        si = inst.sync_info
        waits = list(si.on_wait) if si is not None and si.on_wait else []
        if len(waits) > MAXW and inst.engine != mybir.EngineType.Unassigned:
            rest, keep = waits[:-MAXW], waits[-MAXW:]
            while rest:
                nop = mybir.InstNoOp(
                    name=self.nc.get_next_instruction_name(), ins=[], outs=[]
                )
                nop.engine = inst.engine
                nop.sync_info = bass_rust.SyncInfo(
                    on_wait=rest[:MAXW], on_update=[]
                )
                rest = rest[MAXW:]
                _orig_add(self, nop)
            si.on_wait = keep
        _orig_add(self, inst)

    tile.TileContext._add_instruction = _add_split

    def _patched(self, tick_clock, wait_clock):
        probe = self.nc.sync.nop(nofuse=True)
        wait_clock.add_sem_waits(
            probe.ins, ScopedClock({None: tick_clock.global_clock})
        )
        waits = list(probe.ins.sync_info.on_wait or []) if probe.ins.sync_info else []
        if len(waits) > 1:
            probe.ins.sync_info.on_wait = waits[:1]
            rest = waits[1:]
            while rest:
                n = self.nc.sync.nop(nofuse=True)
                n.ins.sync_info = bass_rust.SyncInfo(on_wait=rest[:1], on_update=[])
                rest = rest[1:]
        self.nc.sync.drain()
        self.nc.all_engine_barrier()
        assert self.sems is not None
        popped = self.nc._tile_sem_poison_stack.pop()
        assert popped is self._sem_poison
        self.nc.clear_and_free_semaphores(list(self.sems.allocated().values()))
        self.nc.all_engine_barrier()

    tile.TileContext._drain_and_barrier = _patched


_install_drain_patch()


def build_nc():
    nc = bass.Bass("TRN2", target_bir_lowering=False, debug=False, num_devices=1)
    xb = nc.dram_tensor("xb", [S, D], F32, kind="ExternalInput").ap()
    wq = nc.dram_tensor("wq", [D, DK], F32, kind="ExternalInput").ap()
    wk = nc.dram_tensor("wk", [D, DK], F32, kind="ExternalInput").ap()
    wv = nc.dram_tensor("wv", [D, DK], F32, kind="ExternalInput").ap()
    bq = nc.dram_tensor("bq", [DK], F32, kind="ExternalInput").ap()
    bk = nc.dram_tensor("bk", [DK], F32, kind="ExternalInput").ap()
    wo = nc.dram_tensor("wo", [DK, D], F32, kind="ExternalInput").ap()
    y = nc.dram_tensor("y", [S, D], F32, kind="ExternalOutput").ap()

    with tile.TileContext(nc, pool_alloc_mode="queue") as tc:
        _emit(nc, tc, xb, wq, wk, wv, bq, bk, wo, y)
    return nc


def _emit(nc, tc, xb, wq, wk, wv, bq, bk, wo, y):
    from contextlib import ExitStack
    from itertools import chain

    ctx = ExitStack()
    with ctx:
        consts = ctx.enter_context(tc.tile_pool(name="consts", bufs=1))
        ident = consts.tile([128, 128], F32, tag="identf32")
        from concourse.masks import make_identity

        make_identity(nc, ident)
        ident_att = ident
        if DT_ATT != F32:
            ident_att = consts.tile([128, 128], DT_ATT, tag="identatt")
            make_identity(nc, ident_att)

        bq_sb = consts.tile([128, NMT], F32, tag="bq")
        bk_sb = consts.tile([128, NMT], F32, tag="bk")
        nc.sync.dma_start(bq_sb[:], bq.rearrange("(m p) -> p m", p=128))
        nc.sync.dma_start(bk_sb[:], bk.rearrange("(m p) -> p m", p=128))

        # ---- persistent tensors ----
        qkv_pool = ctx.enter_context(tc.tile_pool(name="qkv", bufs=1))
        QT = [qkv_pool.tile([128, S], DT_ATT, tag=f"QT{m}", name=f"QT{m}") for m in range(NMT)]
        KT = [qkv_pool.tile([128, S], DT_ATT, tag=f"KT{m}", name=f"KT{m}") for m in range(NMT)]
        VT = qkv_pool.tile([128, NST, 8, 65], DT_ATT, tag="VT")
        nc.vector.memset(VT[:, :, :, 64:65], 1.0)
        upool = ctx.enter_context(tc.tile_pool(name="usb", bufs=1))
        Ut = [upool.tile([128, 8, 64], DT_ATT, tag=f"U{qg}", name=f"U{qg}") for qg in range(NST)]
        utp = ctx.enter_context(tc.tile_pool(name="utp", bufs=1))
        UT = [utp.tile([128, S], DT_ATT, tag=f"UT{k}", name=f"UT{k}") for k in range(NMT)]

        # ---- scope alive through attention: xT, weights, shared psum ----
        with tc.tile_pool(name="xtp", bufs=1) as xtp, \
             tc.tile_pool(name="wst", bufs=1) as wst, \
             tc.tile_pool(name="psx", bufs=2, space="PSUM") as psx:
            xT = [xtp.tile([128, S], DT_PROJ, tag=f"xT{c}", name=f"xT{c}") for c in range(NKC)]

            # A: load + transpose x
            with tc.tile_pool(name="xin", bufs=6) as xinp, \
                 tc.tile_pool(name="wfst", bufs=2) as wfst:
                for stg in range(NST // 4):
                    xins = []
                    for j in range(4):
                        xi = xinp.tile([128, D], F32, tag="xin")
                        nc.sync.dma_start(xi[:], xb[(stg * 4 + j) * 128:(stg * 4 + j + 1) * 128, :])
                        xins.append(xi)
                    for c in range(NKC):
                        pt = psx.tile([128, 4, 128], F32, tag="px", name="ptA")
                        for j in range(4):
                            nc.tensor.transpose(pt[:, j], xins[j][:, c * 128:(c + 1) * 128], ident)
                        nc.vector.tensor_copy(
                            xT[c][:, stg * 512:(stg + 1) * 512].rearrange("p (a b) -> p a b", a=4),
                            pt[:],
                        )

                # stage (and convert) all projection weights
                w_sbs = {}
                for nm, wd in (("wv", wv), ("wq", wq), ("wk", wk)):
                    w_sb = wst.tile([128, NKC, 512], DT_PROJ, tag=f"w{nm}", name=f"w{nm}")
                    if DT_PROJ == F32:
                        for kc in range(NKC):
                            nc.sync.dma_start(w_sb[:, kc], wd[kc * 128:(kc + 1) * 128, :])
                    else:
                        wf = wfst.tile([128, NKC, 512], F32, tag="wf", name="wf")
                        for kc in range(NKC):
                            nc.sync.dma_start(wf[:, kc], wd[kc * 128:(kc + 1) * 128, :])
                        nc.vector.tensor_copy(w_sb[:], wf[:])
                    w_sbs[nm] = w_sb

            # B1: V = x @ wv (natural layout, no bias)
            for st in range(NST):
                pv = psx.tile([128, 512], F32, tag="px", name="pv")
                for kc in range(NKC):
                    nc.tensor.matmul(
                        pv[:], xT[kc][:, st * 128:(st + 1) * 128], w_sbs["wv"][:, kc],
                        start=(kc == 0), stop=(kc == NKC - 1),
                    )
                nc.vector.tensor_copy(
                    VT[:, st, :, 0:64], pv[:].rearrange("p (h d) -> p h d", h=8)
                )

            def proj_gen(p):
                """QT/KT projection for pair p, one instruction per yield."""
                for (w_sb, b_sb, dst) in ((w_sbs["wq"], bq_sb, QT), (w_sbs["wk"], bk_sb, KT)):
                    for nb in range(4):
                        pq = psx.tile([128, 512], F32, tag="px", name="pq")
                        for kc in range(NKC):
                            nc.tensor.matmul(
                                pq[:], w_sb[:, kc, p * 128:(p + 1) * 128],
                                xT[kc][:, nb * 512:(nb + 1) * 512],
                                start=(kc == 0), stop=(kc == NKC - 1),
                            )
                            yield
                        nc.vector.tensor_scalar_add(
                            dst[p][:, nb * 512:(nb + 1) * 512], pq[:], b_sb[:, p:p + 1]
                        )
                        yield

            def ut_gen(p):
                """U -> UT transposes for pair p, one instruction per yield."""
                for qtg in range(4):
                    pt = psx.tile([128, 4, 128], DT_ATT, tag="px", name="ptU")
                    for j in range(4):
                        nc.tensor.transpose(
                            pt[:, j], Ut[qtg * 4 + j][:, 2 * p:2 * p + 2, :], ident_att
                        )
                        yield
                    nc.vector.tensor_copy(
                        UT[p][:, qtg * 512:(qtg + 1) * 512].rearrange("p (a b) -> p a b", a=4),
                        pt[:],
                    )
                    yield

            for _ in proj_gen(0):  # pair 0 upfront
                pass

            # ---- attention, with later pairs' PE-only work interleaved ----
            with tc.tile_pool(name="lps", bufs=2, space="PSUM") as lps, \
                 tc.tile_pool(name="ups", bufs=2, space="PSUM") as ups, \
                 tc.tile_pool(name="epool", bufs=4) as epool, \
                 tc.tile_pool(name="rpool", bufs=4) as rpool:
                for p in range(NMT):
                    gens = []
                    if p < NMT - 1:
                        gens.append(proj_gen(p + 1))
                    if p > 0:
                        gens.append(ut_gen(p - 1))
                    feed = chain(*gens)
                    n_left = (72 if p < NMT - 1 else 0) + (20 if p > 0 else 0)
                    iters_left = 64
                    for qb in range(4):
                        u1 = ups.tile([128, 4, 128], F32, tag="ups", name="u1")
                        u2 = ups.tile([128, 4, 128], F32, tag="ups", name="u2")
                        for kt in range(NST):
                            L = lps.tile([128, 2, 512], F32, tag="L")
                            for half in range(2):
                                hsl = slice(half * 64, (half + 1) * 64)
                                nc.tensor.matmul(
                                    L[:, half],
                                    KT[p][hsl, kt * 128:(kt + 1) * 128],
                                    QT[p][hsl, qb * 512:(qb + 1) * 512],
                                    start=True, stop=True,
                                )
                            E = epool.tile([128, 2, 512], DT_ATT, tag="E")
                            nc.scalar.activation(E[:], L[:], AF.Exp, scale=0.125)
                            for qt in range(4):
                                for half, u in ((0, u1), (1, u2)):
                                    nc.tensor.matmul(
                                        u[:, qt, 0:65],
                                        E[:, half, qt * 128:(qt + 1) * 128],
                                        VT[:, kt, 2 * p + half, :],
                                        start=(kt == 0 and qt == 0),
                                        stop=(kt == NST - 1 and qt == 3),
                                        skip_group_check=True,
                                    )
                            # spread the interleaved feed evenly over kt iters
                            todo = -(-n_left // iters_left) if iters_left else n_left
                            for _ in range(todo):
                                if next(feed, None) is None:
                                    n_left = 0
                                    break
                                n_left -= 1
                            iters_left -= 1
                        for qt in range(4):
                            qg = qb * 4 + qt
                            for half, u in ((0, u1), (1, u2)):
                                r = rpool.tile([128, 1], F32, tag="r")
                                nc.vector.reciprocal(r[:], u[:, qt, 64:65])
                                nc.vector.tensor_scalar_mul(
                                    Ut[qg][:, 2 * p + half, :], u[:, qt, 0:64], r[:]
                                )
                    for _ in feed:  # drain any leftovers
                        pass
                for _ in ut_gen(NMT - 1):  # last pair's transposes
                    pass

        # ---- phase D: y = UT.T @ wo ----
        with tc.tile_pool(name="wop", bufs=1) as wop, \
             tc.tile_pool(name="ystage", bufs=3) as ysp, \
             tc.tile_pool(name="psO", bufs=2, space="PSUM") as pso:
            wo_sb = wop.tile([128, NMT, D], DT_ATT, tag="wo")
            if DT_ATT == F32:
                for kc2 in range(NMT):
                    nc.sync.dma_start(wo_sb[:, kc2], wo[kc2 * 128:(kc2 + 1) * 128, :])
            else:
                wo_f32 = wop.tile([128, NMT, D], F32, tag="wof")
                for kc2 in range(NMT):
                    nc.sync.dma_start(wo_f32[:, kc2], wo[kc2 * 128:(kc2 + 1) * 128, :])
                nc.vector.tensor_copy(wo_sb[:], wo_f32[:])
            for qt in range(NST):
                yp = pso.tile([128, 2, 512], F32, tag="psO")
                for kc2 in range(NMT):
                    for n in range(2):
                        nc.tensor.matmul(
                            yp[:, n],
                            UT[kc2][:, qt * 128:(qt + 1) * 128],
                            wo_sb[:, kc2, n * 512:(n + 1) * 512],
                            start=(kc2 == 0), stop=(kc2 == NMT - 1),
                            skip_group_check=True,
                        )
                ys = ysp.tile([128, D], F32, tag="ys")
                nc.vector.tensor_copy(ys[:].rearrange("p (a b) -> p a b", a=2), yp[:])
                nc.sync.dma_start(y[qt * 128:(qt + 1) * 128, :], ys[:])


_NC_CACHE = None


def _get_nc():
    global _NC_CACHE
    if _NC_CACHE is None:
        _NC_CACHE = build_nc()
    return _NC_CACHE


def kernel(x, wq, bq, wk, bk, wv, bv, wo, bo):
    from concourse.bass_utils import run_bass_kernel_spmd

    x = np.ascontiguousarray(np.asarray(x, np.float32))
    wq, bq = np.asarray(wq, np.float32), np.asarray(bq, np.float32)
    wk, bk = np.asarray(wk, np.float32), np.asarray(bk, np.float32)
    wv, bv = np.asarray(wv, np.float32), np.asarray(bv, np.float32)
    wo, bo = np.asarray(wo, np.float32), np.asarray(bo, np.float32)

    in_maps = []
    for core in range(N_CORES):
        b, hg = core // 2, core % 2
        sl = slice(hg * DK, (hg + 1) * DK)
        in_maps.append({
            "xb": np.ascontiguousarray(x[b]),
            "wq": np.ascontiguousarray(wq[:, sl]),
            "wk": np.ascontiguousarray(wk[:, sl]),
            "wv": np.ascontiguousarray(wv[:, sl]),
            "bq": np.ascontiguousarray(bq[sl]),
            "bk": np.ascontiguousarray(bk[sl]),
            "wo": np.ascontiguousarray(wo[sl, :]),
        })

    nc = _get_nc()
    res = run_bass_kernel_spmd(nc, in_maps, core_ids=list(range(N_CORES)))

    extra = (bo + bv @ wo).astype(np.float32)
    out = np.empty((4, S, D), np.float32)
    for b in range(4):
        out[b] = res.results[2 * b]["y"] + res.results[2 * b + 1]["y"] + extra
    return out


# revision 16
# speedup vs baseline: 3.3653x; 1.0108x over previous
"""Multi-head attention (B=4, S=2048, D=1024, H=16) on 8 TRN2 NeuronCores.

Sharding: batch x head-group (4 batches x 2 groups of 8 heads).  Each core:
  x_b [2048,1024], wq/wk/wv column-slice [1024,512], wo row-slice [512,1024]
  -> partial y [2048,1024]; host sums the two head-group partials per batch
  and adds the folded biases (bo + bv @ wo).

Per-core dataflow (all SBUF-resident, flash-style attention):
  A. x -> xT via PE transposes                      [8 x (128, 2048)]
  B. QT = wq.T @ xT + bq ; KT likewise ; V = x @ wv (+ ones column)
  C. per head-pair, per 512-q block, per 128-k tile:
       L^T = KTh_tile.T @ QTh   (K=64 row-tiled pair, auto tile_position)
       E^T = exp(L^T / 8)       (ACT, scale fused)
       U[q,0:65] += E^T_tile.T @ [V_h | 1]  (PSUM accum over k tiles)
     then U[:,0:64] / U[:,64] -> attention out per head
  D. U -> UT via PE transposes ; y = UT.T @ wo ; DMA out
"""

import numpy as np

from concourse import bass, tile, mybir
from concourse.vector_clock import ScopedClock

F32 = mybir.dt.float32
AF = mybir.ActivationFunctionType

# dtype knobs: (attention operand dtype, projection operand dtype)
DT_ATT = mybir.dt.bfloat16   # QT/KT/V/E/U storage & attention matmul operands
DT_PROJ = mybir.dt.bfloat16  # xT / weight operands for QKV projections

N_CORES = 8
S = 2048          # sequence length per core (one batch)
D = 1024          # d_model
DK = 512          # head-group width (8 heads x 64)
NST = S // 128    # 16 seq tiles
NKC = D // 128    # 8 d_model tiles
NMT = DK // 128   # 4 head-pair tiles


def _install_drain_patch():
    """walrus in this image rejects >1 sync-wait per instruction (the limit
    varies by instruction struct; 1 is always safe).  Spread excess waits
    over preceding same-engine nops: same program point, identical
    semantics, a few ns of sequencer issue overhead."""
    import bass_rust

    MAXW = 1
    _orig_add = tile.TileContext._add_instruction

    def _add_split(self, inst):
        si = inst.sync_info
        waits = list(si.on_wait) if si is not None and si.on_wait else []
        if len(waits) > MAXW and inst.engine != mybir.EngineType.Unassigned:
            rest, keep = waits[:-MAXW], waits[-MAXW:]
            while rest:
                nop = mybir.InstNoOp(
                    name=self.nc.get_next_instruction_name(), ins=[], outs=[]
                )
                nop.engine = inst.engine
                nop.sync_info = bass_rust.SyncInfo(
                    on_wait=rest[:MAXW], on_update=[]
                )
                rest = rest[MAXW:]
                _orig_add(self, nop)
            si.on_wait = keep
        _orig_add(self, inst)

    tile.TileContext._add_instruction = _add_split

    def _patched(self, tick_clock, wait_clock):
        probe = self.nc.sync.nop(nofuse=True)
        wait_clock.add_sem_waits(
            probe.ins, ScopedClock({None: tick_clock.global_clock})
        )
        waits = list(probe.ins.sync_info.on_wait or []) if probe.ins.sync_info else []
        if len(waits) > 1:
            probe.ins.sync_info.on_wait = waits[:1]
            rest = waits[1:]
            while rest:
                n = self.nc.sync.nop(nofuse=True)
                n.ins.sync_info = bass_rust.SyncInfo(on_wait=rest[:1], on_update=[])
                rest = rest[1:]
        self.nc.sync.drain()
        self.nc.all_engine_barrier()
        assert self.sems is not None
        popped = self.nc._tile_sem_poison_stack.pop()
        assert popped is self._sem_poison
        self.nc.clear_and_free_semaphores(list(self.sems.allocated().values()))
        self.nc.all_engine_barrier()

    tile.TileContext._drain_and_barrier = _patched


_install_drain_patch()


def build_nc():
    nc = bass.Bass("TRN2", target_bir_lowering=False, debug=False, num_devices=1)
    xb = nc.dram_tensor("xb", [S, D], F32, kind="ExternalInput").ap()
    wq = nc.dram_tensor("wq", [D, DK], F32, kind="ExternalInput").ap()
    wk = nc.dram_tensor("wk", [D, DK], F32, kind="ExternalInput").ap()
    wv = nc.dram_tensor("wv", [D, DK], F32, kind="ExternalInput").ap()
    bq = nc.dram_tensor("bq", [DK], F32, kind="ExternalInput").ap()
    bk = nc.dram_tensor("bk", [DK], F32, kind="ExternalInput").ap()
    wo = nc.dram_tensor("wo", [DK, D], F32, kind="ExternalInput").ap()
    y = nc.dram_tensor("y", [S, D], F32, kind="ExternalOutput").ap()

    with tile.TileContext(nc, pool_alloc_mode="queue") as tc:
        _emit(nc, tc, xb, wq, wk, wv, bq, bk, wo, y)
    return nc


def _emit(nc, tc, xb, wq, wk, wv, bq, bk, wo, y):
    from contextlib import ExitStack
    from itertools import chain

    ctx = ExitStack()
    with ctx:
        consts = ctx.enter_context(tc.tile_pool(name="consts", bufs=1))
        ident = consts.tile([128, 128], F32, tag="identf32")
        from concourse.masks import make_identity

        make_identity(nc, ident)
        ident_att = ident
        if DT_ATT != F32:
            ident_att = consts.tile([128, 128], DT_ATT, tag="identatt")
            make_identity(nc, ident_att)

        bq_sb = consts.tile([128, NMT], F32, tag="bq")
        bk_sb = consts.tile([128, NMT], F32, tag="bk")
        nc.sync.dma_start(bq_sb[:], bq.rearrange("(m p) -> p m", p=128))
        nc.sync.dma_start(bk_sb[:], bk.rearrange("(m p) -> p m", p=128))

        # ---- persistent tensors ----
        qkv_pool = ctx.enter_context(tc.tile_pool(name="qkv", bufs=1))
        QT = [qkv_pool.tile([128, S], DT_ATT, tag=f"QT{m}", name=f"QT{m}") for m in range(NMT)]
        KT = [qkv_pool.tile([128, S], DT_ATT, tag=f"KT{m}", name=f"KT{m}") for m in range(NMT)]
        VT = qkv_pool.tile([128, NST, 8, 65], DT_ATT, tag="VT")
        nc.vector.memset(VT[:, :, :, 64:65], 1.0)
        upool = ctx.enter_context(tc.tile_pool(name="usb", bufs=1))
        Ut = [upool.tile([128, 8, 64], DT_ATT, tag=f"U{qg}", name=f"U{qg}") for qg in range(NST)]
        utp = ctx.enter_context(tc.tile_pool(name="utp", bufs=1))
        UT = [utp.tile([128, S], DT_ATT, tag=f"UT{k}", name=f"UT{k}") for k in range(NMT)]

        # ---- scope alive through attention: xT, weights, shared psum ----
        with tc.tile_pool(name="xtp", bufs=1) as xtp, \
             tc.tile_pool(name="wst", bufs=1) as wst, \
             tc.tile_pool(name="psx", bufs=2, space="PSUM") as psx:
            xT = [xtp.tile([128, S], DT_PROJ, tag=f"xT{c}", name=f"xT{c}") for c in range(NKC)]

            # A: load + transpose x
            with tc.tile_pool(name="xin", bufs=8) as xinp, \
                 tc.tile_pool(name="wfst", bufs=1) as wfst:
                for stg in range(NST // 4):
                    xins = []
                    for j in range(4):
                        xi = xinp.tile([128, D], F32, tag="xin")
                        nc.sync.dma_start(xi[:], xb[(stg * 4 + j) * 128:(stg * 4 + j + 1) * 128, :])
                        xins.append(xi)
                    for c in range(NKC):
                        pt = psx.tile([128, 4, 128], F32, tag="px", name="ptA")
                        for j in range(4):
                            nc.tensor.transpose(pt[:, j], xins[j][:, c * 128:(c + 1) * 128], ident)
                        nc.vector.tensor_copy(
                            xT[c][:, stg * 512:(stg + 1) * 512].rearrange("p (a b) -> p a b", a=4),
                            pt[:],
                        )

                # stage (and convert) all projection weights
                w_sbs = {}
                for nm, wd in (("wv", wv), ("wq", wq), ("wk", wk)):
                    w_sb = wst.tile([128, NKC, 512], DT_PROJ, tag=f"w{nm}", name=f"w{nm}")
                    if DT_PROJ == F32:
                        for kc in range(NKC):
                            nc.sync.dma_start(w_sb[:, kc], wd[kc * 128:(kc + 1) * 128, :])
                    else:
                        wf = wfst.tile([128, NKC, 512], F32, tag="wf", name="wf")
                        for kc in range(NKC):
                            nc.sync.dma_start(wf[:, kc], wd[kc * 128:(kc + 1) * 128, :])
                        nc.vector.tensor_copy(w_sb[:], wf[:])
                    w_sbs[nm] = w_sb

            # B1: V = x @ wv (natural layout, no bias)
            for st in range(NST):
                pv = psx.tile([128, 512], F32, tag="px", name="pv")
                for kc in range(NKC):
                    nc.tensor.matmul(
                        pv[:], xT[kc][:, st * 128:(st + 1) * 128], w_sbs["wv"][:, kc],
                        start=(kc == 0), stop=(kc == NKC - 1),
                    )
                nc.vector.tensor_copy(
                    VT[:, st, :, 0:64], pv[:].rearrange("p (h d) -> p h d", h=8)
                )

            def proj_gen(p):
                """QT/KT projection for pair p, one instruction per yield."""
                for (w_sb, b_sb, dst) in ((w_sbs["wq"], bq_sb, QT), (w_sbs["wk"], bk_sb, KT)):
                    for nb in range(4):
                        pq = psx.tile([128, 512], F32, tag="px", name="pq")
                        for kc in range(NKC):
                            nc.tensor.matmul(
                                pq[:], w_sb[:, kc, p * 128:(p + 1) * 128],
                                xT[kc][:, nb * 512:(nb + 1) * 512],
                                start=(kc == 0), stop=(kc == NKC - 1),
                            )
                            yield
                        nc.vector.tensor_scalar_add(
                            dst[p][:, nb * 512:(nb + 1) * 512], pq[:], b_sb[:, p:p + 1]
                        )
                        yield

            def ut_gen(p):
                """U -> UT transposes for pair p, one instruction per yield."""
                for qtg in range(4):
                    pt = psx.tile([128, 4, 128], DT_ATT, tag="px", name="ptU")
                    for j in range(4):
                        nc.tensor.transpose(
                            pt[:, j], Ut[qtg * 4 + j][:, 2 * p:2 * p + 2, :], ident_att
                        )
                        yield
                    nc.vector.tensor_copy(
                        UT[p][:, qtg * 512:(qtg + 1) * 512].rearrange("p (a b) -> p a b", a=4),
                        pt[:],
                    )
                    yield

            for _ in proj_gen(0):  # pair 0 upfront
                pass

            # ---- attention, with later pairs' PE-only work interleaved ----
            with tc.tile_pool(name="lps", bufs=2, space="PSUM") as lps, \
                 tc.tile_pool(name="ups", bufs=2, space="PSUM") as ups, \
                 tc.tile_pool(name="epool", bufs=4) as epool, \
                 tc.tile_pool(name="rpool", bufs=4) as rpool:
                for p in range(NMT):
                    gens = []
                    if p < NMT - 1:
                        gens.append(proj_gen(p + 1))
                    if p > 0:
                        gens.append(ut_gen(p - 1))
                    feed = chain(*gens)
                    n_left = (72 if p < NMT - 1 else 0) + (20 if p > 0 else 0)
                    iters_left = 64
                    for qb in range(4):
                        u1 = ups.tile([128, 4, 128], F32, tag="ups", name="u1")
                        u2 = ups.tile([128, 4, 128], F32, tag="ups", name="u2")
                        for kt in range(NST):
                            L = lps.tile([128, 2, 512], F32, tag="L")
                            for half in range(2):
                                hsl = slice(half * 64, (half + 1) * 64)
                                nc.tensor.matmul(
                                    L[:, half],
                                    KT[p][hsl, kt * 128:(kt + 1) * 128],
                                    QT[p][hsl, qb * 512:(qb + 1) * 512],
                                    start=True, stop=True,
                                )
                            E = epool.tile([128, 2, 512], DT_ATT, tag="E")
                            nc.scalar.activation(E[:], L[:], AF.Exp, scale=0.125)
                            for qt in range(4):
                                for half, u in ((0, u1), (1, u2)):
                                    nc.tensor.matmul(
                                        u[:, qt, 0:65],
                                        E[:, half, qt * 128:(qt + 1) * 128],
                                        VT[:, kt, 2 * p + half, :],
                                        start=(kt == 0 and qt == 0),
                                        stop=(kt == NST - 1 and qt == 3),
                                        skip_group_check=True,
                                    )
                            # spread the interleaved feed evenly over kt iters
                            todo = -(-n_left // iters_left) if iters_left else n_left
                            for _ in range(todo):
                                if next(feed, None) is None:
                                    n_left = 0
                                    break
                                n_left -= 1
                            iters_left -= 1
                        for qt in range(4):
                            qg = qb * 4 + qt
                            for half, u in ((0, u1), (1, u2)):
                                r = rpool.tile([128, 1], F32, tag="r")
                                nc.vector.reciprocal(r[:], u[:, qt, 64:65])
                                nc.vector.tensor_scalar_mul(
                                    Ut[qg][:, 2 * p + half, :], u[:, qt, 0:64], r[:]
                                )
                    for _ in feed:  # drain any leftovers
                        pass
                for _ in ut_gen(NMT - 1):  # last pair's transposes
                    pass

        # ---- phase D: y = UT.T @ wo ----
        with tc.tile_pool(name="wop", bufs=1) as wop, \
             tc.tile_pool(name="ystage", bufs=3) as ysp, \
             tc.tile_pool(name="psO", bufs=2, space="PSUM") as pso:
            wo_sb = wop.tile([128, NMT, D], DT_ATT, tag="wo")
            if DT_ATT == F32:
                for kc2 in range(NMT):
                    nc.sync.dma_start(wo_sb[:, kc2], wo[kc2 * 128:(kc2 + 1) * 128, :])
            else:
                wo_f32 = wop.tile([128, NMT, D], F32, tag="wof")
                for kc2 in range(NMT):
                    nc.sync.dma_start(wo_f32[:, kc2], wo[kc2 * 128:(kc2 + 1) * 128, :])
                nc.vector.tensor_copy(wo_sb[:], wo_f32[:])
            for qt in range(NST):
                yp = pso.tile([128, 2, 512], F32, tag="psO")
                for kc2 in range(NMT):
                    for n in range(2):
                        nc.tensor.matmul(
                            yp[:, n],
                            UT[kc2][:, qt * 128:(qt + 1) * 128],
                            wo_sb[:, kc2, n * 512:(n + 1) * 512],
                            start=(kc2 == 0), stop=(kc2 == NMT - 1),
                            skip_group_check=True,
                        )
                ys = ysp.tile([128, D], F32, tag="ys")
                nc.vector.tensor_copy(ys[:].rearrange("p (a b) -> p a b", a=2), yp[:])
                nc.sync.dma_start(y[qt * 128:(qt + 1) * 128, :], ys[:])


_NC_CACHE = None


def _get_nc():
    global _NC_CACHE
    if _NC_CACHE is None:
        _NC_CACHE = build_nc()
    return _NC_CACHE


def kernel(x, wq, bq, wk, bk, wv, bv, wo, bo):
    from concourse.bass_utils import run_bass_kernel_spmd

    x = np.ascontiguousarray(np.asarray(x, np.float32))
    wq, bq = np.asarray(wq, np.float32), np.asarray(bq, np.float32)
    wk, bk = np.asarray(wk, np.float32), np.asarray(bk, np.float32)
    wv, bv = np.asarray(wv, np.float32), np.asarray(bv, np.float32)
    wo, bo = np.asarray(wo, np.float32), np.asarray(bo, np.float32)

    in_maps = []
    for core in range(N_CORES):
        b, hg = core // 2, core % 2
        sl = slice(hg * DK, (hg + 1) * DK)
        in_maps.append({
            "xb": np.ascontiguousarray(x[b]),
            "wq": np.ascontiguousarray(wq[:, sl]),
            "wk": np.ascontiguousarray(wk[:, sl]),
            "wv": np.ascontiguousarray(wv[:, sl]),
            "bq": np.ascontiguousarray(bq[sl]),
            "bk": np.ascontiguousarray(bk[sl]),
            "wo": np.ascontiguousarray(wo[sl, :]),
        })

    nc = _get_nc()
    res = run_bass_kernel_spmd(nc, in_maps, core_ids=list(range(N_CORES)))

    extra = (bo + bv @ wo).astype(np.float32)
    out = np.empty((4, S, D), np.float32)
    for b in range(4):
        out[b] = res.results[2 * b]["y"] + res.results[2 * b + 1]["y"] + extra
    return out
